# revision 75
# baseline (speedup 1.0000x reference)
"""Trainium2 Bass kernel for nn_AutoRegressive — fp8 DoubleRow edition.

Data-parallel over batch (B=2048 -> 256 rows/core, params replicated).
Feature-major on chip: activations are [feature_partition, batch_free].

The dense feedback is fused into the recurrence (W_comb = W_hh + W_ih W_d),
so each step is one [3072x768] @ [768x256] gate matmul plus a small dense
output matmul. Gate matmuls run as fp8e4 DoubleRow (K=256/instruction at
0.5 cyc/row -> 4x bf16 PE rate; measured rel err ~7e-3 vs 2e-2 budget).

Per-step engine plan (measured, ~9.2us/step steady state):
  PE   ~5.8us: 4 waves x (3 pair-bias DoubleRows + 18 k-pair DoubleRows) + dense
  Act  ~7.9us (bottleneck, 83% busy): 4 gate-wave activations [128,1536]
        psum->bf16 planes + tanh(c) in k-pair-aligned chunks
  DVE  ~5.9us: fc, i*g, c+= (fp16), h->fp8 casts, h->bf16, pred copy
Gate psum is organised gate-wise ([128,6,256] f32 = 3 banks/wave) so one Act
instruction covers a whole gate. Biases enter via pair-packed DoubleRow
matmuls against a constant fp8 plane whose j-slot/batch-half pattern routes
two bias vectors per instruction (Act per-partition bias can't vary along
free, and each start=True then initialises exactly one 2KB psum zero-region
in a single write). Cell state c is fp16 (2x DVE); h is materialised twice:
fp8 for the recurrence, bf16 for the dense output (fp8 h on the output path
alone costs ~2.4e-2 rel).

Steady-state idle on Act is the step's two irreducible stalls:
  - cross-step chain (~1.2us): last tanh chunk -> DVE fp8 cast -> kp2
    matmuls -> next sigma-f. PSUM reads are tile-granular (a reader waits
    ALL of a psum tile's accumulating matmuls — probed directly), so
    splitting sigma-f by batch can't unblock it early; instead the last
    k-pair's tail is split asymmetrically at column 170 (F_SLICES) so only
    an 86-column tanh/cast/kp2 piece sits on the chain.
  - psum ring (~0.3us): wave w+2 reuses wave w's 3 psum banks, and one
    1465ns activation window can't cover the 1724ns wave rebuild; 8 banks
    leave no room for a third wave buffer.
Other trims vs the original: dense/pred-copy emitted after all four waves'
matmuls; the final step skips its (unused) fp8 casts and chunks hb so the
exit dense starts early; the PE warm-up is 2 matmuls (the DMA-bound first
steps ramp the p-state anyway); wx lands f-slice-first and wh per-wave so
step 0/1 start as soon as their weights arrive; SBUF pools run 6 buffers
deep so plane/state tile reuse never adds WAR waits.
"""

import sys

sys.path.insert(0, "/opt/trn_rl_repo")

import numpy as np

import concourse.bacc as bacc
import concourse.mybir as mybir
import concourse.tile as tile
from concourse.bass_utils import run_bass_kernel_spmd

UNITS = 768
INPUT_DIM = 96
OUT_STEPS = 64
NCORES = 8
B = 2048
BL = B // NCORES
NU = UNITS // 128  # 6
F32 = mybir.dt.float32
BF16 = mybir.dt.bfloat16
F16 = mybir.dt.float16
FP8 = mybir.dt.float8e4
Sigmoid = mybir.ActivationFunctionType.Sigmoid
Tanh = mybir.ActivationFunctionType.Tanh
MULT = mybir.AluOpType.mult
ADD = mybir.AluOpType.add
DR = mybir.MatmulPerfMode.DoubleRow

SW = 32.0          # weight scale (fp8 headroom)
INV = 1.0 / SW     # dequant on the activation instruction
KAPPA = 2.0        # constant rhs plane for the bias matmul
NBIAS = 16         # partitions carrying bias rows (b*SW/(NBIAS*KAPPA) = b)

# wave order on Act: f, i, g, o  ->  PyTorch gate-block index (i,f,g,o)
WAVE_GATE = [1, 0, 2, 3]
W_F, W_I, W_G, W_O = 0, 1, 2, 3
FUNC = [Sigmoid, Sigmoid, Tanh, Sigmoid]

# tail chunking: c+= / tanh(c) / h casts go out in these unit-slices,
# aligned with the DoubleRow k-pairs so each pair's fp8 cast releases its
# matmuls; equal sizes keep the DVE casts (593ns) in lockstep with the Act
# tanh chunks (612ns)
TAIL = [(0, 2), (2, 4), (4, 6)]
SPLIT = BL // 2  # batch split point for the last k-pair's tail/matmuls
# last k-pair batch slices, asymmetric: the small R piece is the only work
# on the cross-step critical chain (swept; 170 balances the L-slice cast
# finishing before the R tanh lands against the R chain's own length)
F_SLICES = [(0, 170), (170, BL)]

_prog_cache = {}


def _build_program(steps=OUT_STEPS):
    if steps in _prog_cache:
        return _prog_cache[steps]

    nc = bacc.Bacc("TRN2", target_bir_lowering=False, debug=False, num_devices=NCORES)
    x0_ext = nc.declare_dram_parameter("x0", [128, BL], BF16, isOutput=False)
    wx_ext = nc.declare_dram_parameter("wx", [128, 4 * UNITS], BF16, isOutput=False)
    # bias rows pair-packed: j-slot 0 -> even chunk's bias, 1 -> odd chunk's
    wb_ext = nc.declare_dram_parameter("wb", [128, 2, 2 * UNITS], FP8, isOutput=False)
    wh_ext = nc.declare_dram_parameter("wh", [128, 3, 2, 4 * UNITS], FP8, isOutput=False)
    wd_ext = nc.declare_dram_parameter("wd", [128, NU + 1, 128], BF16, isOutput=False)
    out_ext = nc.declare_dram_parameter("out", [steps, INPUT_DIM, BL], F32, isOutput=True)

    with tile.TileContext(nc) as tc:
        with (
            tc.tile_pool(name="const", bufs=1) as const,
            tc.tile_pool(name="state", bufs=6) as state,
            tc.tile_pool(name="work", bufs=6) as work,
            tc.tile_pool(name="psw", bufs=2, space="PSUM") as psw,
            tc.tile_pool(name="psd", bufs=2, space="PSUM") as psd,
        ):
            x0 = const.tile([128, BL], BF16, tag="x0")
            nc.sync.dma_start(x0[:], x0_ext[:])
            wx = const.tile([128, 4 * UNITS], BF16, tag="wx")
            # f-wave slice first so the first x0-wave starts earliest
            nc.sync.dma_start(wx[:, :UNITS], wx_ext[:, :UNITS])
            nc.sync.dma_start(wx[:, UNITS:], wx_ext[:, UNITS:])
            wb = const.tile([128, 2, 2 * UNITS], FP8, tag="wb")
            nc.sync.dma_start(wb[:], wb_ext[:])
            wh = const.tile([128, 3, 2, 4 * UNITS], FP8, tag="wh")
            # wave-major DMA slices: wave f's weights land first, so step 1
            # starts ~5us earlier instead of stalling on the full 2.25MB
            wd = const.tile([128, NU + 1, 128], BF16, tag="wd")
            for w in range(4):
                nc.sync.dma_start(
                    wh[:, :, :, w * UNITS : (w + 1) * UNITS],
                    wh_ext[:, :, :, w * UNITS : (w + 1) * UNITS],
                )
                if w == 1:
                    # dense weights between the i and g slices: dense(0) needs
                    # them before the o-wave weights are needed
                    nc.sync.dma_start(wd[:], wd_ext[:])

            # kap8[j, n]: kappa on (j=0, n<BL) and (j=1, n>=BL), else 0 — the
            # j-slot/batch-half pattern routes two bias vectors through one
            # [128,512]-wide DoubleRow matmul
            kap8 = const.tile([128, 2, 2 * BL], FP8, tag="kap8")
            nc.vector.memset(kap8[:, 0, :BL], KAPPA)
            nc.vector.memset(kap8[:, 0, BL:], 0.0)
            nc.vector.memset(kap8[:, 1, :BL], 0.0)
            nc.vector.memset(kap8[:, 1, BL:], KAPPA)
            ones16 = const.tile([128, BL], BF16, tag="ones16")
            nc.vector.memset(ones16[:], 1.0)

            # PE p-state ramp on throwaway matmuls while weights stream in
            warm = psd.tile([128, 2 * BL], F32, tag="pred", name="warm")
            for _ in range(2):
                nc.tensor.matmul(
                    warm[:, :BL], x0[:, :128], x0[:], start=True, stop=True,
                    skip_group_check=True,
                )

            def msl(w, u):
                m = w * 6 + u
                return slice(m * 128, (m + 1) * 128)

            def new_step(t):
                return {
                    "t": t,
                    "waves": [None] * 4,
                    "planes": [
                        work.tile([128, NU, BL], BF16, tag=f"pl{w}", name=f"pl{w}_{t}")
                        for w in range(4)
                    ],
                    "tct": work.tile([128, NU, BL], BF16, tag="tct", name=f"tct_{t}"),
                    "m1": None,  # allocated in tail() for t > 0 only
                    "c": state.tile([128, NU, BL], F16, tag="c", name=f"c_{t}"),
                    "h8": state.tile([128, NU, BL], FP8, tag="h8", name=f"h8_{t}"),
                    "hb": state.tile([128, NU, BL], BF16, tag="hb", name=f"hb_{t}"),
                    "pred": None,
                }

            def mm_wave_x0(st, w):
                """Step-0 wave: bf16 x0 matmuls (bias rides x0's ones-row)."""
                wave = psw.tile([128, NU, BL], F32, tag="wave", name=f"wv{w}_0")
                st["waves"][w] = wave
                for u in range(NU):
                    nc.tensor.matmul(
                        wave[:, u], wx[:, msl(w, u)], x0[:], start=True, stop=True
                    )

            def mm_wave(st, w, h8_prev):
                """Steady wave: per chunk one bias DoubleRow + 3 k-pair DoubleRows.

                PSUM start=True lazily invalidates the chunk's whole 2KB
                zero-region (2 chunks), so only the region's FIRST write may
                carry start; the odd chunk's first write lands on pending
                bytes and overwrites, which is exactly its first
                contribution. stop goes on the region's last write.
                """
                t = st["t"]
                wave = psw.tile([128, NU, BL], F32, tag="wave", name=f"wv{w}_{t}")
                st["waves"][w] = wave
                for pr in range(NU // 2):
                    m = w * 3 + pr
                    nc.tensor.matmul(
                        wave[:, 2 * pr : 2 * pr + 2],
                        wb[:, :, m * 128 : (m + 1) * 128], kap8[:],
                        start=True, stop=False, perf_mode=DR,
                        skip_group_check=True,
                    )
                for kp in range(2):
                    for u in range(NU):
                        nc.tensor.matmul(
                            wave[:, u], wh[:, kp, :, msl(w, u)],
                            h8_prev[:, 2 * kp : 2 * kp + 2],
                            start=False, stop=False,
                            perf_mode=DR, skip_group_check=True,
                        )
                if w == W_F:
                    # kp2 split by batch (asymmetric): the small R slice is
                    # the only work gated by the very last tanh/cast chunk,
                    # shortening the cross-step tail before this wave's
                    # activation
                    for h0, h1 in F_SLICES:
                        for u in range(NU):
                            nc.tensor.matmul(
                                wave[:, u, h0:h1], wh[:, 2, :, msl(w, u)],
                                h8_prev[:, 4:6, h0:h1],
                                start=False,
                                stop=(h1 == BL and u % 2 == 1),
                                perf_mode=DR, skip_group_check=True,
                            )
                else:
                    for u in range(NU):
                        nc.tensor.matmul(
                            wave[:, u], wh[:, 2, :, msl(w, u)],
                            h8_prev[:, 4:6],
                            start=False, stop=(u % 2 == 1),
                            perf_mode=DR, skip_group_check=True,
                        )

            def act_wave(st, w):
                scale = 1.0 if st["t"] == 0 else INV
                nc.scalar.activation(
                    st["planes"][w][:], st["waves"][w][:], FUNC[w], scale=scale
                )
                st["waves"][w] = None

            def dense(st):
                """Deferred dense output for step st (pred = W_d h + b_d on PE)."""
                t = st["t"]
                # [128, 512] so each buffer owns a full 2KB zero-region
                ps = psd.tile([128, 2 * BL], F32, tag="pred", name=f"pred_{t}")
                st["pred"] = ps
                for k in range(NU):
                    nc.tensor.matmul(
                        ps[:, :BL], wd[:, k], st["hb"][:, k], start=(k == 0), stop=False
                    )
                nc.tensor.matmul(ps[:, :BL], wd[:, NU], ones16[:], start=False, stop=True)

            def finalize(st, split=False):
                pred = work.tile([INPUT_DIM, BL], F32, tag="predsb", name=f"po_{st['t']}")
                if split:
                    # exit path: copy/DMA in halves so the first transfer
                    # overlaps the last tanh/dense slice
                    for h0, h1 in ((0, SPLIT), (SPLIT, BL)):
                        nc.vector.tensor_copy(
                            pred[:, h0:h1], st["pred"][:INPUT_DIM, h0:h1]
                        )
                        nc.sync.dma_start(out_ext[st["t"], :, h0:h1], pred[:, h0:h1])
                else:
                    nc.vector.tensor_copy(pred[:], st["pred"][:INPUT_DIM, :BL])
                    nc.sync.dma_start(out_ext[st["t"]], pred[:])

            def tail(st, c_prev):
                """c update + tanh(c) + h casts, chunked for the cross-step cycle."""
                t = st["t"]
                pf, pi, pg, po = st["planes"]
                c, tct = st["c"], st["tct"]
                if t == 0:
                    nc.vector.tensor_tensor(c[:], pi[:], pg[:], MULT)
                else:
                    m1 = work.tile([128, NU, BL], F16, tag="m1", name=f"m1_{t}")
                    nc.vector.tensor_tensor(c[:], pf[:], c_prev[:], MULT)
                    # m1 chunked so c+ (and the tanh behind it) starts right
                    # after the g-wave activation instead of a full plane later
                    for a, b in TAIL:
                        nc.vector.tensor_tensor(m1[:, a:b], pi[:, a:b], pg[:, a:b], MULT)
                        nc.vector.tensor_tensor(c[:, a:b], c[:, a:b], m1[:, a:b], ADD)
                if st["t"] == steps - 1:
                    # final step: no fp8 casts needed; hb rides right behind
                    # each tanh chunk so the exit dense starts early, and the
                    # last slice keeps only a small cast on the exit path
                    for a, b in TAIL:
                        if b == NU:
                            for h0, h1 in F_SLICES:
                                nc.scalar.activation(
                                    tct[:, a:b, h0:h1], c[:, a:b, h0:h1], Tanh
                                )
                                nc.vector.tensor_tensor(
                                    st["hb"][:, a:b, h0:h1],
                                    po[:, a:b, h0:h1], tct[:, a:b, h0:h1], MULT,
                                )
                        else:
                            nc.scalar.activation(tct[:, a:b], c[:, a:b], Tanh)
                            nc.vector.tensor_tensor(
                                st["hb"][:, a:b], po[:, a:b], tct[:, a:b], MULT
                            )
                    return
                for a, b in TAIL:
                    if b == NU:
                        # last k-pair in batch halves: tct-L/cast-L overlap
                        # tct-R on Act, so the next wave's kp2-L matmuls
                        # start while the R half is still cooking
                        for h0, h1 in F_SLICES:
                            nc.scalar.activation(
                                tct[:, a:b, h0:h1], c[:, a:b, h0:h1], Tanh
                            )
                            nc.vector.tensor_tensor(
                                st["h8"][:, a:b, h0:h1],
                                po[:, a:b, h0:h1], tct[:, a:b, h0:h1], MULT,
                            )
                    else:
                        nc.scalar.activation(tct[:, a:b], c[:, a:b], Tanh)
                        nc.vector.tensor_tensor(
                            st["h8"][:, a:b], po[:, a:b], tct[:, a:b], MULT
                        )
                nc.vector.tensor_tensor(st["hb"][:], po[:], tct[:], MULT)

            # ---- emission ----------------------------------------------------
            prev = None
            for t in range(steps):
                st = new_step(t)
                if t == 0:
                    for w in range(4):
                        mm_wave_x0(st, w)
                        act_wave(st, w)
                    tail(st, None)
                    prev = st
                    continue

                h8p, cp = prev["h8"], prev["c"]
                mm_wave(st, W_F, h8p)
                act_wave(st, W_F)
                mm_wave(st, W_I, h8p)
                act_wave(st, W_I)
                mm_wave(st, W_G, h8p)
                act_wave(st, W_G)
                mm_wave(st, W_O, h8p)
                # previous step's dense + output after all four waves' matmuls
                # so the g/o waves aren't queued behind it on PE
                dense(prev)
                act_wave(st, W_O)
                finalize(prev)
                tail(st, cp)
                prev = st

            # PE sat idle through the last tail; re-warm the p-state ramp on
            # junk matmuls gated by the same hb chunk the exit dense needs,
            # so the dense runs at mid rather than low p-state
            kw = psd.tile([128, 2 * BL], F32, tag="pred", name="kw")
            for _ in range(2):
                nc.tensor.matmul(
                    kw[:, :BL], x0[:, :128], prev["hb"][:, 0], start=True,
                    stop=True, skip_group_check=True,
                )
            dense(prev)
            finalize(prev)

    nc.compile()
    _prog_cache[steps] = nc
    return nc


def _prep_inputs(inputs, W_ih, W_hh, b_ih, b_hh, W_d, b_d):
    import ml_dtypes

    U, I = UNITS, INPUT_DIM
    W_ih = np.asarray(W_ih, np.float64)
    W_hh = np.asarray(W_hh, np.float64)
    W_d = np.asarray(W_d, np.float64)
    b_ih = np.asarray(b_ih, np.float64)
    b_hh = np.asarray(b_hh, np.float64)
    b_d = np.asarray(b_d, np.float64)

    W_comb_T = (W_hh + W_ih @ W_d).T  # [U, 4U]
    b_comb = b_ih + b_hh + W_ih @ b_d  # [4U]

    # column permutation: position m*128+r (m = wave*6+u) <- gate*768+u*128+r
    perm = np.empty(4 * U, dtype=np.int64)
    for m in range(24):
        w, u = m // 6, m % 6
        src = WAVE_GATE[w] * U + u * 128
        perm[m * 128 : (m + 1) * 128] = np.arange(src, src + 128)

    A = (W_comb_T * SW)[:, perm]  # [768, 3072]
    wh = np.ascontiguousarray(
        A.reshape(3, 2, 128, 4 * U).transpose(2, 0, 1, 3)
    ).astype(ml_dtypes.float8_e4m3)

    # pair-packed bias rows: column block m = w*3+pr holds the biases of
    # chunks (w*6+2pr) in j-slot 0 and (w*6+2pr+1) in j-slot 1
    bp = b_comb[perm].reshape(24, 128)
    wb = np.zeros((128, 2, 2 * U), dtype=np.float32)
    for w in range(4):
        for pr in range(3):
            m = w * 3 + pr
            wb[:NBIAS, 0, m * 128 : (m + 1) * 128] = bp[w * 6 + 2 * pr][None, :]
            wb[:NBIAS, 1, m * 128 : (m + 1) * 128] = bp[w * 6 + 2 * pr + 1][None, :]
    wb = wb.astype(ml_dtypes.float8_e4m3)

    wx = np.zeros((128, 4 * U), dtype=np.float32)
    wx[:I] = W_ih.T[:, perm]
    wx[I] = (b_ih + b_hh)[perm]
    wx = wx.astype(ml_dtypes.bfloat16)

    wd = np.zeros((128, NU + 1, 128), dtype=np.float32)
    wd[:, :NU, :I] = W_d.T.reshape(NU, 128, I).transpose(1, 0, 2)
    wd[0, NU, :I] = b_d
    wd = wd.astype(ml_dtypes.bfloat16)

    x_last = np.asarray(inputs[:, -1, :], dtype=np.float32)  # [B, I]
    in_maps = []
    for c in range(NCORES):
        x0 = np.zeros((128, BL), dtype=np.float32)
        x0[:I] = x_last[c * BL : (c + 1) * BL].T
        x0[I] = 1.0
        x0 = x0.astype(ml_dtypes.bfloat16)
        in_maps.append({"x0": x0, "wx": wx, "wb": wb, "wh": wh, "wd": wd})
    return in_maps


def kernel(inputs, W_ih, W_hh, b_ih, b_hh, W_d, b_d):
    in_maps = _prep_inputs(np.asarray(inputs), W_ih, W_hh, b_ih, b_hh, W_d, b_d)
    nc = _build_program()
    res = run_bass_kernel_spmd(nc, in_maps, core_ids=list(range(NCORES)))
    parts = [np.transpose(res.results[c]["out"], (2, 0, 1)) for c in range(NCORES)]
    return np.ascontiguousarray(np.concatenate(parts, axis=0))

